# revision 5
# baseline (speedup 1.0000x reference)
"""Trainium2 Bass kernel for nn_AnalyticalMNet.

Reference computation (per batch b of B=64):
    C       : (2, HW)   concentrations (flattened C_mean)
    Y       : (HW, 3)   optical-density pixels
    gram    = C @ C^T                                  (2,2)
    A       = (1/sigma_sq) I + (1/lambda_sq) gram      (2,2)
    out_var = A^{-1}                                   (2,2)
    yc[s,n] = sum_p Y[p,s] C[n,p]                      (3,2)
    U       = (1/sigma_sq) M_ref + (1/lambda_sq) yc    (3,2)
    out_mean= U @ out_var                              (3,2)

Strategy: pure data parallel over 8 NeuronCores (8 batches per core).
Per batch all heavy work is 9 dot-products over HW=262144 elements:
  3 gram entries + 6 yc entries.
On-device per core:
  - DMA C (128 x 2 x 2048) and Y (128 x 2048 x 3) tiles per batch.
  - 2 ACT passes: activation(Square, accum_out) -> sum C0^2, sum C1^2.
  - 7 DVE passes: scalar_tensor_tensor(mult, mult, accum_out) fused
    multiply+reduce -> C0*C1 and the six C_n*Y_s products.
    (per-partition partial sums, free-axis reduced)
  - One PE matmul against a ones(128,1) stationary reduces the
    (128, 72) partials across partitions -> (1, 72) sums.
  - Tiny DVE epilogue computes the 2x2 inverse + affine update for all
    8 batches vectorized along the free axis; DMA out (1, 80) per core.
"""

import numpy as np

import concourse.bass as bass
import concourse.tile as tile
from concourse import bacc, mybir
from concourse.bass_utils import run_bass_kernel_spmd

F32 = mybir.dt.float32
AOP = mybir.AluOpType
ACTF = mybir.ActivationFunctionType

B = 64
N_CORES = 8
BPC = B // N_CORES          # batches per core
HW = 512 * 512              # pixels per batch
P = 128                     # SBUF partitions


def build_kernel(bpc=BPC, hw=HW):
    """Build the per-core SPMD Bass graph. Returns the Bass object."""
    J = hw // P             # free-dim columns per partition

    nc = bacc.Bacc()
    C_ext = nc.declare_dram_parameter("C", [bpc, 2, hw], F32, isOutput=False)
    Y_ext = nc.declare_dram_parameter("Y", [bpc, hw, 3], F32, isOutput=False)
    M_ext = nc.declare_dram_parameter("M", [1, 6 * bpc], F32, isOutput=False)
    S_ext = nc.declare_dram_parameter("S", [1, 2], F32, isOutput=False)
    O_ext = nc.declare_dram_parameter("out", [1, 10 * bpc], F32, isOutput=True)

    NB = 9 * bpc            # number of big dot products
    OFFV = 6 * bpc          # out_var offset inside the output row

    with tile.TileContext(nc) as tc:
        with (
            tc.tile_pool(name="const", bufs=1) as const,
            tc.tile_pool(name="data", bufs=3) as data,
            tc.tile_pool(name="prod", bufs=2) as prod,
            tc.tile_pool(name="psum", bufs=1, space="PSUM") as psum,
        ):
            ones = const.tile([P, 1], F32)
            nc.vector.memset(ones[:], 1.0)
            accums = const.tile([P, NB], F32)
            m_t = const.tile([1, 6 * bpc], F32)
            nc.sync.dma_start(m_t[:], M_ext[:])
            sl_t = const.tile([1, 2], F32)
            nc.sync.dma_start(sl_t[:], S_ext[:])

            for b in range(bpc):
                c_t = data.tile([P, 2, J], F32, tag="c")
                nc.sync.dma_start(
                    c_t[:], C_ext[b].rearrange("n (p j) -> p n j", p=P)
                )
                y_t = data.tile([P, J, 3], F32, tag="y")
                nc.sync.dma_start(
                    y_t[:], Y_ext[b].rearrange("(p j) s -> p j s", p=P)
                )
                c0 = c_t[:, 0, :]
                c1 = c_t[:, 1, :]
                base = 9 * b
                # gram diagonal on the scalar engine (fused square+reduce)
                sq0 = prod.tile([P, J], F32, tag="sq")
                nc.scalar.activation(
                    sq0[:], c0, ACTF.Square,
                    accum_out=accums[:, base + 0 : base + 1],
                )
                sq1 = prod.tile([P, J], F32, tag="sq")
                nc.scalar.activation(
                    sq1[:], c1, ACTF.Square,
                    accum_out=accums[:, base + 2 : base + 3],
                )
                # gram off-diagonal + yc on the vector engine (fused mult+reduce)
                pr = prod.tile([P, J], F32, tag="pr")
                nc.vector.scalar_tensor_tensor(
                    pr[:], c0, 1.0, c1, op0=AOP.mult, op1=AOP.mult,
                    accum_out=accums[:, base + 1 : base + 2],
                )
                for s in range(3):
                    for n in range(2):
                        col = base + 3 + s * 2 + n
                        pr = prod.tile([P, J], F32, tag="pr")
                        nc.vector.scalar_tensor_tensor(
                            pr[:], c_t[:, n, :], 1.0, y_t[:, :, s],
                            op0=AOP.mult, op1=AOP.mult,
                            accum_out=accums[:, col : col + 1],
                        )

            # cross-partition reduction: ones^T @ accums -> (1, NB)
            sums_p = psum.tile([1, NB], F32)
            nc.tensor.matmul(sums_p[:], ones[:], accums[:], start=True, stop=True)
            sums_t = const.tile([1, NB], F32)
            nc.vector.tensor_copy(sums_t[:], sums_p[:])

            # ---- epilogue: 2x2 inverse + affine update, vectorized over b ----
            out_t = const.tile([1, 10 * bpc], F32)
            recips = const.tile([1, 2], F32)
            nc.vector.reciprocal(recips[:], sl_t[:])
            isg = recips[:, 0:1]       # 1/sigma_sq
            ilq = recips[:, 1:2]       # 1/lambda_sq

            w_t = const.tile([1, NB], F32)
            nc.vector.tensor_scalar_mul(w_t[:], sums_t[:], ilq)
            wv = w_t[:].rearrange("p (b k) -> p b k", b=bpc)
            g00 = wv[:, :, 0]
            g01 = wv[:, :, 1]
            g11 = wv[:, :, 2]

            e_a = const.tile([1, bpc], F32)
            nc.vector.tensor_scalar_add(e_a[:], g00, isg)
            e_d = const.tile([1, bpc], F32)
            nc.vector.tensor_scalar_add(e_d[:], g11, isg)
            t_ad = const.tile([1, bpc], F32)
            nc.vector.tensor_mul(t_ad[:], e_a[:], e_d[:])
            t_b2 = const.tile([1, bpc], F32)
            nc.vector.tensor_mul(t_b2[:], g01, g01)
            det = const.tile([1, bpc], F32)
            nc.vector.tensor_sub(det[:], t_ad[:], t_b2[:])
            rdet = const.tile([1, bpc], F32)
            nc.vector.reciprocal(rdet[:], det[:])

            # out_var = adj(A) * rdet, stored [v00 v01 v01 v11] per batch
            vv = out_t[:, OFFV:].rearrange("p (b k) -> p b k", k=4)
            nc.vector.tensor_mul(vv[:, :, 0], e_d[:], rdet[:])
            nc.vector.scalar_tensor_tensor(
                vv[:, :, 1], g01, -1.0, rdet[:], op0=AOP.mult, op1=AOP.mult
            )
            nc.vector.tensor_copy(vv[:, :, 2], vv[:, :, 1])
            nc.vector.tensor_mul(vv[:, :, 3], e_a[:], rdet[:])

            # U = isg*M_ref + ilq*yc   (layout (b, s, n) -> 6 per batch)
            u_t = const.tile([1, 6 * bpc], F32)
            uv = u_t[:].rearrange("p (b k) -> p b k", b=bpc)
            mv = m_t[:].rearrange("p (b k) -> p b k", b=bpc)
            nc.vector.scalar_tensor_tensor(
                uv[:, :, :], mv[:, :, :], isg, wv[:, :, 3:9],
                op0=AOP.mult, op1=AOP.add,
            )

            # out_mean[b,s,n] = U[b,s,0]*V[b,0,n] + U[b,s,1]*V[b,1,n]
            u4 = u_t[:].rearrange("p (b s n) -> p b s n", b=bpc, s=3)
            t1 = const.tile([1, 6 * bpc], F32)
            t1v = t1[:].rearrange("p (b s n) -> p b s n", b=bpc, s=3)
            t2 = const.tile([1, 6 * bpc], F32)
            t2v = t2[:].rearrange("p (b s n) -> p b s n", b=bpc, s=3)
            shp = (1, bpc, 3, 2)

            def _bcast_mid(ap_in, n):
                # insert a stride-0 dim of size n before the last dim
                dims = [list(d) for d in ap_in.ap]
                dims.insert(len(dims) - 1, [0, n])
                return bass.AP(ap_in.tensor, ap_in.offset, dims)

            nc.vector.tensor_mul(
                t1v[:, :, :, :],
                u4[:, :, :, 0:1].broadcast_to(shp),
                _bcast_mid(vv[:, :, 0:2], 3),
            )
            nc.vector.tensor_mul(
                t2v[:, :, :, :],
                u4[:, :, :, 1:2].broadcast_to(shp),
                _bcast_mid(vv[:, :, 1:4:2], 3),
            )
            nc.vector.tensor_add(out_t[:, 0:OFFV], t1[:], t2[:])

            nc.sync.dma_start(O_ext[:], out_t[:])

    nc.compile()
    return nc


def _postprocess(rows, bpc):
    """rows: list of per-core (1, 10*bpc) results -> (out_mean, out_var)."""
    means, varis = [], []
    for r in rows:
        r = np.asarray(r).reshape(-1)
        means.append(r[: 6 * bpc].reshape(bpc, 3, 2))
        varis.append(r[6 * bpc :].reshape(bpc, 2, 2))
    return np.concatenate(means, 0), np.concatenate(varis, 0)


def make_in_maps(Y_OD, C_mean, M_ref, sigma_sq, lambda_sq, bpc=BPC, n_cores=N_CORES):
    Y_OD = np.ascontiguousarray(np.asarray(Y_OD, dtype=np.float32))
    C_mean = np.ascontiguousarray(np.asarray(C_mean, dtype=np.float32))
    M_ref = np.ascontiguousarray(np.asarray(M_ref, dtype=np.float32))
    hw = Y_OD.shape[1]
    sl = np.array([[np.float32(sigma_sq), np.float32(lambda_sq)]], dtype=np.float32)
    in_maps = []
    for i in range(n_cores):
        lo, hi = i * bpc, (i + 1) * bpc
        in_maps.append(
            {
                "C": C_mean[lo:hi].reshape(bpc, 2, hw),
                "Y": Y_OD[lo:hi],
                "M": M_ref[lo:hi].reshape(1, 6 * bpc),
                "S": sl,
            }
        )
    return in_maps


_NC_CACHE = {}


def _get_nc(bpc, hw):
    key = (bpc, hw)
    if key not in _NC_CACHE:
        _NC_CACHE[key] = build_kernel(bpc, hw)
    return _NC_CACHE[key]


def kernel(Y_OD, C_mean, M_ref, sigma_sq, lambda_sq, trace=False, **trace_kwargs):
    nc = _get_nc(BPC, HW)
    in_maps = make_in_maps(Y_OD, C_mean, M_ref, sigma_sq, lambda_sq)
    res = run_bass_kernel_spmd(
        nc, in_maps, core_ids=list(range(N_CORES)), trace=trace, **trace_kwargs
    )
    out_mean, out_var = _postprocess([m["out"] for m in res.results], BPC)
    if trace:
        kernel.last_exec_time_ns = res.exec_time_ns
        kernel.last_results = res
    return out_mean, out_var
